# revision 17
# baseline (speedup 1.0000x reference)
"""GCNII (32-layer) on 8 Trainium2 NeuronCores via Bass/Tile.

Design (see notes.md):
- Nodes sharded 6250/core (padded to 6272 = 49*128). Per layer the scaled
  table H~ = dinv * H is AllGathered to a DRAM table; each core gathers its
  in-edge source rows with dma_gather (4 SWDGE queues, 4-way split per
  gather), aggregates via DVE adds in an ELL k-major layout (2 src-halves x
  16 slots; slot 15 references per-(core,half) "virtual nodes" that
  pre-aggregate overflow edges of high-degree nodes), then a dense tail
  (PE transpose + matmul with P_l = (1-b)I + bW folded on host) and relu.
- AH[n] = dinv[n] * (H~[n] + sum_{dst(e)=n} H~[src_e]);  H~ = dinv * H.
"""

import os
import hashlib
import numpy as np

import concourse.bacc as bacc
import concourse.mybir as mybir
import concourse.tile as tile
from concourse import library_config
from concourse.bass_utils import run_bass_kernel_spmd

N = 50000
E = 1600000
C = 8                    # cores
RPC = 6250               # real nodes per core
NPC = 6272               # padded nodes per core (49*128)
NB = NPC // 128          # 49 blocks
F = 64                   # hidden
FIN = 512                # input feature dim
LAYERS = int(os.environ.get("GCN_LAYERS", "32"))
ABLATE = set(os.environ.get("GCN_ABLATE", "").split(","))
LAMBDA = 0.5
ALPHA = 0.1
HALF = 4 * NPC           # 25088 rows per half in table
ZROW = RPC               # view-local always-zero row (first pad row)
KM = 16                  # main ELL slots per half (15 real + 1 vnode ref)
BSPLIT = [0, 12, 24, 36, NB]   # 4-way block split for gathers

_CACHE = {}


# ---------------------------------------------------------------- host prep

def _wrap_idx(a):
    """flat [n] (n % 16 == 0) -> [128, n/16] int16 SWDGE layout."""
    n = a.shape[0]
    return np.tile(a.reshape(n // 16, 16).T, (8, 1)).astype(np.int16)


def _preprocess(src, dst):
    """Build per-core index structures. Returns (meta, per_core_arrays)."""
    deg = np.bincount(dst, minlength=N).astype(np.float32) + 1.0
    dinv = (1.0 / np.sqrt(deg)).astype(np.float32)

    sc = src // RPC
    srow_all = (sc % 4) * NPC + src % RPC      # view-local row of each src
    shalf_all = sc // 4

    cores = []
    for c in range(C):
        m = (dst >= c * RPC) & (dst < (c + 1) * RPC)
        ed = (dst[m] - c * RPC).astype(np.int64)
        halves = []
        for h in (0, 1):
            hm = shalf_all[m] == h
            d_h = ed[hm]
            r_h = srow_all[m][hm].astype(np.int32)
            order = np.argsort(d_h, kind="stable")
            d_s = d_h[order]
            r_s = r_h[order]
            cnt = np.bincount(d_s, minlength=RPC).astype(np.int64)
            ptr = np.zeros(RPC + 1, np.int64)
            np.cumsum(cnt, out=ptr[1:])
            halves.append((cnt, ptr, r_s))
        cores.append(halves)

    # global structure params
    vcounts = []     # per (c,h) number of vnodes
    kvs = []
    for c in range(C):
        for h in (0, 1):
            cnt = cores[c][h][0]
            vn = cnt[cnt >= KM]
            vcounts.append(len(vn))
            kvs.append(int(vn.max() - (KM - 1)) if len(vn) else 0)
    VPC = int(np.ceil((max(vcounts) + 1) / 128) * 128)
    KV = max(kvs)
    # per-tier counts (max over cores), rounded to 128
    nvk = []
    for kp in range(KV):
        mx = 0
        for c in range(C):
            for h in (0, 1):
                cnt = cores[c][h][0]
                mx = max(mx, int(np.count_nonzero(cnt >= KM + kp)))
        nvk.append(int(np.ceil(max(mx, 1) / 128) * 128))
    ZV = VPC - 1
    vcols = [n // 16 for n in nvk]
    voff = np.concatenate([[0], np.cumsum(vcols)]).astype(np.int64)
    VCOLS = int(voff[-1])

    per_core = []
    for c in range(C):
        idx_main = np.zeros((128, 2, KM, NPC // 16), np.int16)
        idx_v = np.zeros((128, 2, max(VCOLS, 1)), np.int16)
        for h in (0, 1):
            cnt, ptr, r_s = cores[c][h]
            for k in range(KM - 1):
                arr = np.full(NPC, ZROW, np.int32)
                valid = cnt > k
                arr[:RPC][valid] = r_s[ptr[:-1][valid] + k]
                idx_main[:, h, k, :] = _wrap_idx(arr)
            # vnodes sorted by vdeg desc
            vn_nodes = np.nonzero(cnt >= KM)[0]
            vdeg = (cnt[vn_nodes] - (KM - 1)).astype(np.int64)
            vorder = np.argsort(-vdeg, kind="stable")
            vn_nodes = vn_nodes[vorder]
            vdeg = vdeg[vorder]
            # slot 15 -> vnode id (in vnode view) or ZV
            arr = np.full(NPC, ZV, np.int32)
            arr[:RPC][vn_nodes] = np.arange(len(vn_nodes))
            idx_main[:, h, KM - 1, :] = _wrap_idx(arr)
            # vnode tiers
            for kp in range(KV):
                nk = nvk[kp]
                cnt_k = int(np.count_nonzero(vdeg > kp))
                arr = np.full(nk, ZROW, np.int32)
                if cnt_k:
                    nodes_k = vn_nodes[:cnt_k]
                    arr[:cnt_k] = r_s[ptr[nodes_k] + (KM - 1) + kp]
                idx_v[:, h, voff[kp]:voff[kp + 1]] = _wrap_idx(arr)
        # dinv tiles
        dpad = np.zeros(NPC, np.float32)
        dpad[:RPC] = dinv[c * RPC:(c + 1) * RPC]
        dinv_t = dpad.reshape(NB, 128).T.copy()          # [128, 49]
        per_core.append(dict(idx_main=idx_main, idx_v=idx_v,
                             dinv_t=dinv_t,
                             dinv1_t=((1.0 - ALPHA) * dinv_t).copy()))

    meta = dict(VPC=VPC, KV=KV, nvk=nvk, voff=voff, VCOLS=max(VCOLS, 1), ZV=ZV)
    return meta, per_core


# ---------------------------------------------------------------- program

def _build(meta):
    VPC, KV, nvk, voff = meta["VPC"], meta["KV"], meta["nvk"], meta["voff"]
    VCOLS = meta["VCOLS"]
    NVB = VPC // 128
    TROWS = 2 * HALF + 2 * VPC

    nc = bacc.Bacc(None, num_swdge_queues=4)
    dt = mybir.dt
    f32 = dt.float32

    # external inputs (per-core)
    featT_in = nc.dram_tensor("featT", [FIN, NPC], f32, kind="ExternalInput")
    w0_in = nc.dram_tensor("w0t", [128, 4, F], f32, kind="ExternalInput")
    b0_in = nc.dram_tensor("b0t", [128, F], f32, kind="ExternalInput")
    pall_in = nc.dram_tensor("pall", [F, LAYERS * F], f32, kind="ExternalInput")
    wl_in = nc.dram_tensor("wlt", [F, F], f32, kind="ExternalInput")
    bl_in = nc.dram_tensor("blt", [128, F], f32, kind="ExternalInput")
    dinv_in = nc.dram_tensor("dinv_t", [128, NB], f32, kind="ExternalInput")
    dinv1_in = nc.dram_tensor("dinv1_t", [128, NB], f32, kind="ExternalInput")
    im_in = nc.dram_tensor("idx_main", [128, 2, KM, NPC // 16], dt.int16, kind="ExternalInput")
    iv_in = nc.dram_tensor("idx_v", [128, 2, VCOLS], dt.int16, kind="ExternalInput")
    ident_in = nc.dram_tensor("ident", [128, 128], f32, kind="ExternalInput")
    out_t = nc.dram_tensor("out_t", [128, NB, F], f32, kind="ExternalOutput")

    with tile.TileContext(nc) as tc:
        nc.gpsimd.load_library(library_config.mlp)
        with (
            tc.tile_pool(name="dram", bufs=1, space="DRAM") as dram,
            tc.tile_pool(name="dramsh", bufs=LAYERS, space="DRAM") as dramsh,
            tc.tile_pool(name="const", bufs=1) as cst,
            tc.tile_pool(name="state", bufs=1) as st,
            tc.tile_pool(name="msg", bufs=4) as msgp,
            tc.tile_pool(name="vmsg", bufs=2) as vmsgp,
            tc.tile_pool(name="ft", bufs=2) as ftp,
            tc.tile_pool(name="hnt", bufs=2) as hntp,
            tc.tile_pool(name="ps", bufs=1, space="PSUM") as psp,
            tc.tile_pool(name="pst", bufs=1, space="PSUM") as pstp,
        ):
            table = dram.tile([TROWS, F], f32)
            agin = dram.tile([NPC, F], f32)

            viewV = [table[2 * HALF:2 * HALF + VPC, :],
                     table[2 * HALF + VPC:2 * HALF + 2 * VPC, :]]

            # residents
            im = cst.tile([128, 2, KM, NPC // 16], dt.int16)
            nc.sync.dma_start(im[:], im_in[:])
            iv = cst.tile([128, 2, VCOLS], dt.int16)
            nc.sync.dma_start(iv[:], iv_in[:])
            dinv_t = cst.tile([128, NB], f32)
            nc.sync.dma_start(dinv_t[:], dinv_in[:])
            dinv1_t = cst.tile([128, NB], f32)
            nc.sync.dma_start(dinv1_t[:], dinv1_in[:])
            w0s = cst.tile([128, 4, F], f32)
            nc.sync.dma_start(w0s[:], w0_in[:])
            b0t = cst.tile([128, F], f32)
            nc.sync.dma_start(b0t[:], b0_in[:])
            blt = cst.tile([128, F], f32)
            nc.sync.dma_start(blt[:], bl_in[:])
            pall = cst.tile([F, LAYERS * F], f32)
            nc.sync.dma_start(pall[:], pall_in[:])
            wlt = cst.tile([F, F], f32)
            nc.sync.dma_start(wlt[:], wl_in[:])
            ident = cst.tile([128, 128], f32)
            nc.sync.dma_start(ident[:], ident_in[:])

            Ht = st.tile([128, NB, F], f32)       # current H (node-major)
            H0s = st.tile([128, NB, F], f32)      # alpha * H0
            Htil = st.tile([128, NB, F], f32)     # H~ = dinv * H
            acc = st.tile([128, NB, F], f32)
            Hn = st.tile([128, NB, F], f32)
            HnTf = st.tile([F, NB * 128], f32)
            vacc0 = st.tile([128, NVB, F], f32)
            vacc1 = st.tile([128, NVB, F], f32)
            vacc = [vacc0, vacc1]

            PS = psp.tile([128, NB, F], f32)      # 6.125 banks

            # ---------------- input projection: H0 = relu(feat @ W0 + b0)
            for b in range(NB):
                ft = ftp.tile([128, 4, 128], f32)
                nc.sync.dma_start(
                    ft[:], featT_in[:, b * 128:(b + 1) * 128]
                    .rearrange("(q p) n -> p q n", p=128))
                for q in range(4):
                    nc.tensor.matmul(PS[:, b, :], ft[:, q, :], w0s[:, q, :],
                                     start=(q == 0), stop=(q == 3))
            b0b = b0t[:].rearrange("p (o f) -> p o f", o=1).to_broadcast([128, NB, F])
            nc.vector.tensor_add(Ht[:], PS[:], b0b)
            nc.vector.tensor_scalar_max(Ht[:], Ht[:], 0.0)
            nc.vector.tensor_scalar_mul(H0s[:], Ht[:], ALPHA)

            qrot = [0]

            def gathers(msg_ap, view, idx_ap, nidx, split4):
                """Emit gather(s) for one (half, k)."""
                if split4:
                    for j in range(4):
                        b0_, b1_ = BSPLIT[j], BSPLIT[j + 1]
                        nq = (b1_ - b0_) * 128
                        nc.gpsimd.dma_gather(
                            msg_ap[:, b0_:b1_, :], view,
                            idx_ap[:, b0_ * 8:b1_ * 8],
                            nq, nq, F, single_packet=False, queue_num=j)
                else:
                    nc.gpsimd.dma_gather(
                        msg_ap, view, idx_ap, nidx, nidx, F,
                        single_packet=False, queue_num=qrot[0] & 3)
                    qrot[0] += 1

            # ---------------- layers
            for l in range(LAYERS):
                # H~ = H * dinv
                nc.vector.tensor_mul(Htil[:], Ht[:],
                                     dinv_t[:].to_broadcast([128, NB, F]))
                # AG input bounce (node-major)
                nc.sync.dma_start(
                    agin[:].rearrange("(b p) f -> p b f", p=128), Htil[:])
                tsh = dramsh.tile([2 * HALF, F], f32, addr_space="Shared",
                                  name="tsh", tag="tsh")
                viewA = tsh[0:HALF, :]
                viewB = tsh[HALF:2 * HALF, :]
                if "noag" not in ABLATE:
                    nc.gpsimd.collective_compute(
                        "AllGather", mybir.AluOpType.bypass,
                        ins=[agin[:]], outs=[tsh[:]],
                        replica_groups=[list(range(C))],
                    )

                # main aggregation, slots 0..KM-2 (vnode-independent)
                nc.vector.tensor_copy(acc[:], Htil[:])
                for h in ((0, 1) if "noagg" not in ABLATE else ()):
                    view = viewA if h == 0 else viewB
                    for k in range(KM - 1):
                        msg = msgp.tile([128, NB, F], f32)
                        gathers(msg[:], view, im[:, h, k, :], NPC, True)
                        if "noadd" not in ABLATE:
                            for j in range(4):
                                b0_, b1_ = BSPLIT[j], BSPLIT[j + 1]
                                nc.vector.tensor_add(
                                    acc[:, b0_:b1_, :], acc[:, b0_:b1_, :],
                                    msg[:, b0_:b1_, :])

                # vnode pre-aggregation (overlaps tail of main gathers)
                for h in ((0, 1) if "nov" not in ABLATE else ()):
                    nc.vector.memset(vacc[h][:], 0.0)
                    view = viewA if h == 0 else viewB
                    for kp in range(KV):
                        nk = nvk[kp]
                        nb = nk // 128
                        vm = vmsgp.tile([128, NVB, F], f32)
                        if nb >= 8:
                            sp = [0, nb // 4, nb // 2, 3 * nb // 4, nb]
                            for j in range(4):
                                nq = (sp[j + 1] - sp[j]) * 128
                                nc.gpsimd.dma_gather(
                                    vm[:, sp[j]:sp[j + 1], :], view,
                                    iv[:, h, voff[kp] + sp[j] * 8: voff[kp] + sp[j + 1] * 8],
                                    nq, nq, F, single_packet=False, queue_num=j)
                        else:
                            nc.gpsimd.dma_gather(
                                vm[:, :nb, :], view,
                                iv[:, h, voff[kp]:voff[kp + 1]],
                                nk, nk, F, single_packet=False,
                                queue_num=qrot[0] & 3)
                            qrot[0] += 1
                        nc.vector.tensor_add(vacc[h][:, :nb, :],
                                             vacc[h][:, :nb, :], vm[:, :nb, :])
                    # write vnode sums into table
                    nc.sync.dma_start(
                        viewV[h].rearrange("(b p) f -> p b f", p=128), vacc[h][:])

                # slot KM-1: vnode references (after viewV written)
                for h in ((0, 1) if "noagg" not in ABLATE else ()):
                    msg = msgp.tile([128, NB, F], f32)
                    gathers(msg[:], viewV[h], im[:, h, KM - 1, :], NPC, True)
                    if "noadd" not in ABLATE:
                        nc.vector.tensor_add(acc[:], acc[:], msg[:])

                # tail: Hn = (1-a)*dinv*acc + a*H0 ; H = relu(Hn @ P_l)
                nc.vector.tensor_mul(Hn[:], acc[:],
                                     dinv1_t[:].to_broadcast([128, NB, F]))
                nc.vector.tensor_add(Hn[:], Hn[:], H0s[:])
                if "notail" not in ABLATE:
                    for g in range(0, NB, 4):
                        gl = min(4, NB - g)
                        psT4 = pstp.tile([F, 4, 128], f32)
                        for j in range(gl):
                            nc.tensor.transpose(psT4[:, j, :], Hn[:, g + j, :],
                                                ident[:])
                        nc.vector.tensor_copy(
                            HnTf[:, g * 128:(g + gl) * 128],
                            psT4[:, :gl, :])
                    for b in range(NB):
                        nc.tensor.matmul(PS[:, b, :],
                                         HnTf[:, b * 128:(b + 1) * 128],
                                         pall[:, l * F:(l + 1) * F],
                                         start=True, stop=True)
                    nc.vector.tensor_scalar_max(Ht[:], PS[:], 0.0)
                else:
                    nc.vector.tensor_scalar_max(Ht[:], Hn[:], 0.0)

            # ---------------- final: out = H @ Wl + bl
            for g in range(0, NB, 4):
                gl = min(4, NB - g)
                psT4 = pstp.tile([F, 4, 128], f32)
                for j in range(gl):
                    nc.tensor.transpose(psT4[:, j, :], Ht[:, g + j, :], ident[:])
                nc.vector.tensor_copy(HnTf[:, g * 128:(g + gl) * 128],
                                      psT4[:, :gl, :])
            for b in range(NB):
                nc.tensor.matmul(PS[:, b, :],
                                 HnTf[:, b * 128:(b + 1) * 128], wlt[:],
                                 start=True, stop=True)
            blb = blt[:].rearrange("p (o f) -> p o f", o=1).to_broadcast([128, NB, F])
            nc.vector.tensor_add(Hn[:], PS[:], blb)
            nc.sync.dma_start(out_t[:], Hn[:])

    nc.compile()
    return nc


# ---------------------------------------------------------------- entry

def _get_program(src, dst):
    key = hashlib.sha256(src.tobytes() + dst.tobytes()).hexdigest()[:16]
    key = (key, LAYERS, tuple(sorted(ABLATE)))
    if key not in _CACHE:
        meta, per_core = _preprocess(np.asarray(src, np.int64),
                                     np.asarray(dst, np.int64))
        nc = _build(meta)
        _CACHE[key] = (meta, per_core, nc)
    return _CACHE[key]


def _in_maps(per_core, feature, W0, b0, Wc, Wl, bl):
    ls = np.arange(1, LAYERS + 1, dtype=np.float32)
    beta = np.log(LAMBDA / ls + 1.0)
    pall = np.zeros((F, LAYERS * F), np.float32)
    eye = np.eye(F, dtype=np.float32)
    for l in range(LAYERS):
        pall[:, l * F:(l + 1) * F] = (1.0 - beta[l]) * eye + beta[l] * Wc[l]
    w0t = np.ascontiguousarray(
        W0.reshape(4, 128, F).transpose(1, 0, 2)).astype(np.float32)
    b0t = np.tile(b0[None, :], (128, 1)).astype(np.float32)
    blt = np.tile(bl[None, :], (128, 1)).astype(np.float32)
    ident = np.eye(128, dtype=np.float32)
    maps = []
    for c in range(C):
        featT = np.zeros((FIN, NPC), np.float32)
        featT[:, :RPC] = feature[c * RPC:(c + 1) * RPC].T
        pc = per_core[c]
        maps.append(dict(featT=featT, w0t=w0t, b0t=b0t, pall=pall,
                         wlt=np.ascontiguousarray(Wl, dtype=np.float32),
                         blt=blt, dinv_t=pc["dinv_t"], dinv1_t=pc["dinv1_t"],
                         idx_main=pc["idx_main"], idx_v=pc["idx_v"],
                         ident=ident))
    return maps


def kernel(feature, W0, b0, Wc, Wl, bl, src, dst):
    feature = np.asarray(feature, np.float32)
    meta, per_core, nc = _get_program(np.asarray(src), np.asarray(dst))
    maps = _in_maps(per_core, feature, np.asarray(W0, np.float32),
                    np.asarray(b0, np.float32), np.asarray(Wc, np.float32),
                    np.asarray(Wl, np.float32), np.asarray(bl, np.float32))
    res = run_bass_kernel_spmd(nc, maps, core_ids=list(range(C)))
    out = np.empty((N, F), np.float32)
    for c in range(C):
        o = res.results[c]["out_t"]          # [128, NB, F]
        out[c * RPC:(c + 1) * RPC] = o.transpose(1, 0, 2).reshape(NPC, F)[:RPC]
    return out



# revision 26
# speedup vs baseline: 1.4445x; 1.4445x over previous
"""GCNII (32-layer) on 8 Trainium2 NeuronCores via Bass/Tile.

Design (see notes.md):
- Nodes sharded 6250/core (padded to 6272 = 49*128). Per layer the scaled
  table H~ = dinv * H is AllGathered to a DRAM table; each core gathers its
  in-edge source rows with dma_gather (4 SWDGE queues, 4-way split per
  gather), aggregates via DVE adds in an ELL k-major layout (2 src-halves x
  16 slots; slot 15 references per-(core,half) "virtual nodes" that
  pre-aggregate overflow edges of high-degree nodes), then a dense tail
  (PE transpose + matmul with P_l = (1-b)I + bW folded on host) and relu.
- AH[n] = dinv[n] * (H~[n] + sum_{dst(e)=n} H~[src_e]);  H~ = dinv * H.
"""

import os
import hashlib
import numpy as np

import concourse.bacc as bacc
import concourse.mybir as mybir
import concourse.tile as tile
from concourse import library_config
from concourse.bass_utils import run_bass_kernel_spmd

N = 50000
E = 1600000
C = 8                    # cores
RPC = 6250               # real nodes per core
NPC = 6272               # padded nodes per core (49*128)
NB = NPC // 128          # 49 blocks
F = 64                   # hidden
FIN = 512                # input feature dim
LAYERS = int(os.environ.get("GCN_LAYERS", "32"))
ABLATE = set(os.environ.get("GCN_ABLATE", "").split(","))
LAMBDA = 0.5
ALPHA = 0.1
HALF = 4 * NPC           # 25088 rows per half in table
ZROW = RPC               # view-local always-zero row (first pad row)
KM = 16                  # main ELL slots per half (15 real + 1 vnode ref)
BSPLIT = [0, 12, 24, 36, NB]   # 4-way block split for gathers

_CACHE = {}


# ---------------------------------------------------------------- host prep

def _wrap_idx(a):
    """flat [n] (n % 16 == 0) -> [128, n/16] int16 SWDGE layout."""
    n = a.shape[0]
    return np.tile(a.reshape(n // 16, 16).T, (8, 1)).astype(np.int16)


def _preprocess(src, dst):
    """Build per-core index structures. Returns (meta, per_core_arrays)."""
    deg = np.bincount(dst, minlength=N).astype(np.float32) + 1.0
    dinv = (1.0 / np.sqrt(deg)).astype(np.float32)

    sc = src // RPC
    srow_all = (sc % 4) * NPC + src % RPC      # view-local row of each src
    shalf_all = sc // 4

    cores = []
    for c in range(C):
        m = (dst >= c * RPC) & (dst < (c + 1) * RPC)
        ed = (dst[m] - c * RPC).astype(np.int64)
        halves = []
        for h in (0, 1):
            hm = shalf_all[m] == h
            d_h = ed[hm]
            r_h = srow_all[m][hm].astype(np.int32)
            order = np.argsort(d_h, kind="stable")
            d_s = d_h[order]
            r_s = r_h[order]
            cnt = np.bincount(d_s, minlength=RPC).astype(np.int64)
            ptr = np.zeros(RPC + 1, np.int64)
            np.cumsum(cnt, out=ptr[1:])
            halves.append((cnt, ptr, r_s))
        cores.append(halves)

    # global structure params
    vcounts = []     # per (c,h) number of vnodes
    kvs = []
    for c in range(C):
        for h in (0, 1):
            cnt = cores[c][h][0]
            vn = cnt[cnt >= KM]
            vcounts.append(len(vn))
            kvs.append(int(vn.max() - (KM - 1)) if len(vn) else 0)
    VPC = int(np.ceil((max(vcounts) + 1) / 128) * 128)
    KV = max(kvs)
    # per-tier counts (max over cores), rounded to 128
    nvk = []
    for kp in range(KV):
        mx = 0
        for c in range(C):
            for h in (0, 1):
                cnt = cores[c][h][0]
                mx = max(mx, int(np.count_nonzero(cnt >= KM + kp)))
        nvk.append(int(np.ceil(max(mx, 1) / 128) * 128))
    ZV = VPC - 1
    vcols = [n // 16 for n in nvk]
    voff = np.concatenate([[0], np.cumsum(vcols)]).astype(np.int64)
    VCOLS = int(voff[-1])

    zspread = (ZROW + (np.arange(NPC) % (NPC - RPC))).astype(np.int32)
    mv = max(vcounts)
    zvspread = (mv + (np.arange(NPC) % (VPC - mv))).astype(np.int32)
    per_core = []
    for c in range(C):
        idx_main = np.zeros((128, 2, KM, NPC // 16), np.int16)
        idx_v = np.zeros((128, 2, max(VCOLS, 1)), np.int16)
        for h in (0, 1):
            cnt, ptr, r_s = cores[c][h]
            for k in range(KM - 1):
                arr = zspread.copy()
                valid = cnt > k
                arr[:RPC][valid] = r_s[ptr[:-1][valid] + k]
                idx_main[:, h, k, :] = _wrap_idx(arr)
            # vnodes sorted by vdeg desc
            vn_nodes = np.nonzero(cnt >= KM)[0]
            vdeg = (cnt[vn_nodes] - (KM - 1)).astype(np.int64)
            vorder = np.argsort(-vdeg, kind="stable")
            vn_nodes = vn_nodes[vorder]
            vdeg = vdeg[vorder]
            # slot 15 -> vnode id, or a zero row of the vnode view
            arr = zvspread.copy()
            arr[:RPC][vn_nodes] = np.arange(len(vn_nodes))
            idx_main[:, h, KM - 1, :] = _wrap_idx(arr)
            # vnode tiers
            for kp in range(KV):
                nk = nvk[kp]
                cnt_k = int(np.count_nonzero(vdeg > kp))
                arr = zspread[:nk].copy()
                if cnt_k:
                    nodes_k = vn_nodes[:cnt_k]
                    arr[:cnt_k] = r_s[ptr[nodes_k] + (KM - 1) + kp]
                idx_v[:, h, voff[kp]:voff[kp + 1]] = _wrap_idx(arr)
        # dinv tiles
        dpad = np.zeros(NPC, np.float32)
        dpad[:RPC] = dinv[c * RPC:(c + 1) * RPC]
        dinv_t = dpad.reshape(NB, 128).T.copy()          # [128, 49]
        per_core.append(dict(idx_main=idx_main, idx_v=idx_v,
                             dinv_t=dinv_t,
                             dinv1_t=((1.0 - ALPHA) * dinv_t).copy()))

    meta = dict(VPC=VPC, KV=KV, nvk=nvk, voff=voff, VCOLS=max(VCOLS, 1), ZV=ZV)
    return meta, per_core


# ---------------------------------------------------------------- program

def _build(meta):
    VPC, KV, nvk, voff = meta["VPC"], meta["KV"], meta["nvk"], meta["voff"]
    VCOLS = meta["VCOLS"]
    NVB = VPC // 128
    TROWS = 2 * HALF + 2 * VPC

    nc = bacc.Bacc(None, num_swdge_queues=4)
    dt = mybir.dt
    f32 = dt.float32

    # external inputs (per-core)
    featT_in = nc.dram_tensor("featT", [FIN, NPC], f32, kind="ExternalInput")
    w0_in = nc.dram_tensor("w0t", [128, 4, F], f32, kind="ExternalInput")
    b0_in = nc.dram_tensor("b0t", [128, F], f32, kind="ExternalInput")
    pall_in = nc.dram_tensor("pall", [F, LAYERS * F], f32, kind="ExternalInput")
    wl_in = nc.dram_tensor("wlt", [F, F], f32, kind="ExternalInput")
    bl_in = nc.dram_tensor("blt", [128, F], f32, kind="ExternalInput")
    dinv_in = nc.dram_tensor("dinv_t", [128, NB], f32, kind="ExternalInput")
    dinv1_in = nc.dram_tensor("dinv1_t", [128, NB], f32, kind="ExternalInput")
    im_in = nc.dram_tensor("idx_main", [128, 2, KM, NPC // 16], dt.int16, kind="ExternalInput")
    iv_in = nc.dram_tensor("idx_v", [128, 2, VCOLS], dt.int16, kind="ExternalInput")
    ident_in = nc.dram_tensor("ident", [128, 128], f32, kind="ExternalInput")
    out_t = nc.dram_tensor("out_t", [128, NB, F], f32, kind="ExternalOutput")

    with tile.TileContext(nc) as tc:
        nc.gpsimd.load_library(library_config.mlp)
        with (
            tc.tile_pool(name="dram", bufs=1, space="DRAM") as dram,
            tc.tile_pool(name="dramsh", bufs=LAYERS, space="DRAM") as dramsh,
            tc.tile_pool(name="const", bufs=1) as cst,
            tc.tile_pool(name="state", bufs=1) as st,
            tc.tile_pool(name="msg", bufs=5) as msgp,
            tc.tile_pool(name="ft", bufs=2) as ftp,
            tc.tile_pool(name="hnt", bufs=2) as hntp,
            tc.tile_pool(name="ps", bufs=1, space="PSUM") as psp,
            tc.tile_pool(name="pst", bufs=1, space="PSUM") as pstp,
        ):
            table = dram.tile([TROWS, F], f32)
            agin = dram.tile([NPC, F], f32)

            viewV = [table[2 * HALF:2 * HALF + VPC, :],
                     table[2 * HALF + VPC:2 * HALF + 2 * VPC, :]]

            # residents
            im = cst.tile([128, 2, KM, NPC // 16], dt.int16)
            nc.sync.dma_start(im[:], im_in[:])
            iv = cst.tile([128, 2, VCOLS], dt.int16)
            nc.sync.dma_start(iv[:], iv_in[:])
            dinv_t = cst.tile([128, NB], f32)
            nc.sync.dma_start(dinv_t[:], dinv_in[:])
            dinv1_t = cst.tile([128, NB], f32)
            nc.sync.dma_start(dinv1_t[:], dinv1_in[:])
            w0s = cst.tile([128, 4, F], f32)
            nc.sync.dma_start(w0s[:], w0_in[:])
            b0t = cst.tile([128, F], f32)
            nc.sync.dma_start(b0t[:], b0_in[:])
            blt = cst.tile([128, F], f32)
            nc.sync.dma_start(blt[:], bl_in[:])
            pall = cst.tile([F, LAYERS * F], f32)
            nc.sync.dma_start(pall[:], pall_in[:])
            wlt = cst.tile([F, F], f32)
            nc.sync.dma_start(wlt[:], wl_in[:])
            ident = cst.tile([128, 128], f32)
            nc.sync.dma_start(ident[:], ident_in[:])

            Ht = st.tile([128, NB, F], f32)       # current H (node-major)
            H0s = st.tile([128, NB, F], f32)      # alpha * H0
            Htil = st.tile([128, NB, F], f32)     # H~ = dinv * H
            acc = st.tile([128, NB, F], f32)
            Hn = st.tile([128, NB, F], f32)
            HnTf = st.tile([F, NB * 128], f32)
            vacc0 = st.tile([128, NVB, F], f32)
            vacc1 = st.tile([128, NVB, F], f32)
            vacc = [vacc0, vacc1]

            PS = psp.tile([128, NB, F], f32)      # 6.125 banks

            # ---------------- input projection: H0 = relu(feat @ W0 + b0)
            for b in range(NB):
                ft = ftp.tile([128, 4, 128], f32)
                nc.sync.dma_start(
                    ft[:], featT_in[:, b * 128:(b + 1) * 128]
                    .rearrange("(q p) n -> p q n", p=128))
                for q in range(4):
                    nc.tensor.matmul(PS[:, b, :], ft[:, q, :], w0s[:, q, :],
                                     start=(q == 0), stop=(q == 3))
            b0b = b0t[:].rearrange("p (o f) -> p o f", o=1).to_broadcast([128, NB, F])
            nc.vector.tensor_add(Ht[:], PS[:], b0b)
            nc.vector.tensor_scalar_max(Ht[:], Ht[:], 0.0)
            nc.vector.tensor_scalar_mul(H0s[:], Ht[:], ALPHA)

            qrot = [0]

            def gathers(msg_ap, view, idx_ap, nidx, split4):
                """Emit gather(s) for one (half, k)."""
                if split4:
                    for j in range(4):
                        b0_, b1_ = BSPLIT[j], BSPLIT[j + 1]
                        nq = (b1_ - b0_) * 128
                        nc.gpsimd.dma_gather(
                            msg_ap[:, b0_:b1_, :], view,
                            idx_ap[:, b0_ * 8:b1_ * 8],
                            nq, nq, F, single_packet=False, queue_num=j)
                else:
                    nc.gpsimd.dma_gather(
                        msg_ap, view, idx_ap, nidx, nidx, F,
                        single_packet=False, queue_num=qrot[0] & 3)
                    qrot[0] += 1

            # ---------------- layers
            for l in range(LAYERS):
                # H~ = H * dinv
                nc.vector.tensor_mul(Htil[:], Ht[:],
                                     dinv_t[:].to_broadcast([128, NB, F]))
                # AG input bounce (node-major)
                nc.sync.dma_start(
                    agin[:].rearrange("(b p) f -> p b f", p=128), Htil[:])
                tsh = dramsh.tile([2 * HALF, F], f32, addr_space="Shared",
                                  name="tsh", tag="tsh")
                viewA = tsh[0:HALF, :]
                viewB = tsh[HALF:2 * HALF, :]
                if "noag" not in ABLATE:
                    nc.gpsimd.collective_compute(
                        "AllGather", mybir.AluOpType.bypass,
                        ins=[agin[:]], outs=[tsh[:]],
                        replica_groups=[list(range(C))],
                    )

                # main aggregation, slots 0..KM-2 (vnode-independent)
                nc.vector.tensor_copy(acc[:], Htil[:])
                for h in ((0, 1) if "noagg" not in ABLATE else ()):
                    view = viewA if h == 0 else viewB
                    for k in range(KM - 1):
                        msg = msgp.tile([128, NB, F], f32)
                        gathers(msg[:], view, im[:, h, k, :], NPC, True)
                        if "noadd" not in ABLATE:
                            for j in range(4):
                                b0_, b1_ = BSPLIT[j], BSPLIT[j + 1]
                                nc.vector.tensor_add(
                                    acc[:, b0_:b1_, :], acc[:, b0_:b1_, :],
                                    msg[:, b0_:b1_, :])

                # vnode pre-aggregation (overlaps tail of main gathers):
                # tiers merged into <=NB-block gather groups (msgp tiles)
                for h in ((0, 1) if "nov" not in ABLATE else ()):
                    nc.vector.memset(vacc[h][:], 0.0)
                    view = viewA if h == 0 else viewB
                    # build groups of consecutive tiers totaling <= NB blocks
                    groups = []
                    t0 = 0
                    blks = 0
                    for kp in range(KV):
                        b = nvk[kp] // 128
                        if blks + b > NB and blks > 0:
                            groups.append((t0, kp))
                            t0, blks = kp, 0
                        blks += b
                    if KV:
                        groups.append((t0, KV))
                    for (ta, tb) in groups:
                        c0, c1 = int(voff[ta]), int(voff[tb])
                        nq = (c1 - c0) * 16
                        gb = nq // 128
                        vm = msgp.tile([128, NB, F], f32, name="msg")
                        nc.gpsimd.dma_gather(
                            vm[:, :gb, :], view, iv[:, h, c0:c1],
                            nq, nq, F, single_packet=False,
                            queue_num=qrot[0] & 3)
                        qrot[0] += 1
                        goff = 0
                        for kp in range(ta, tb):
                            nb = nvk[kp] // 128
                            nc.vector.tensor_add(
                                vacc[h][:, :nb, :], vacc[h][:, :nb, :],
                                vm[:, goff:goff + nb, :])
                            goff += nb
                    # write vnode sums into table
                    nc.sync.dma_start(
                        viewV[h].rearrange("(b p) f -> p b f", p=128), vacc[h][:])

                # slot KM-1: vnode references (after viewV written)
                for h in ((0, 1) if "noagg" not in ABLATE else ()):
                    msg = msgp.tile([128, NB, F], f32)
                    gathers(msg[:], viewV[h], im[:, h, KM - 1, :], NPC, True)
                    if "noadd" not in ABLATE:
                        nc.vector.tensor_add(acc[:], acc[:], msg[:])

                # tail: Hn = (1-a)*dinv*acc + a*H0 ; H = relu(Hn @ P_l)
                nc.vector.tensor_mul(Hn[:], acc[:],
                                     dinv1_t[:].to_broadcast([128, NB, F]))
                nc.vector.tensor_add(Hn[:], Hn[:], H0s[:])
                if "notail" not in ABLATE:
                    for g in range(0, NB, 4):
                        gl = min(4, NB - g)
                        psT4 = pstp.tile([F, 4, 128], f32)
                        for j in range(gl):
                            nc.tensor.transpose(psT4[:, j, :], Hn[:, g + j, :],
                                                ident[:])
                        nc.vector.tensor_copy(
                            HnTf[:, g * 128:(g + gl) * 128],
                            psT4[:, :gl, :])
                    for b in range(NB):
                        nc.tensor.matmul(PS[:, b, :],
                                         HnTf[:, b * 128:(b + 1) * 128],
                                         pall[:, l * F:(l + 1) * F],
                                         start=True, stop=True)
                    nc.vector.tensor_scalar_max(Ht[:], PS[:], 0.0)
                else:
                    nc.vector.tensor_scalar_max(Ht[:], Hn[:], 0.0)

            # ---------------- final: out = H @ Wl + bl
            for g in range(0, NB, 4):
                gl = min(4, NB - g)
                psT4 = pstp.tile([F, 4, 128], f32)
                for j in range(gl):
                    nc.tensor.transpose(psT4[:, j, :], Ht[:, g + j, :], ident[:])
                nc.vector.tensor_copy(HnTf[:, g * 128:(g + gl) * 128],
                                      psT4[:, :gl, :])
            for b in range(NB):
                nc.tensor.matmul(PS[:, b, :],
                                 HnTf[:, b * 128:(b + 1) * 128], wlt[:],
                                 start=True, stop=True)
            blb = blt[:].rearrange("p (o f) -> p o f", o=1).to_broadcast([128, NB, F])
            nc.vector.tensor_add(Hn[:], PS[:], blb)
            nc.sync.dma_start(out_t[:], Hn[:])

    nc.compile()
    return nc


# ---------------------------------------------------------------- entry

def _get_program(src, dst):
    key = hashlib.sha256(src.tobytes() + dst.tobytes()).hexdigest()[:16]
    key = (key, LAYERS, tuple(sorted(ABLATE)))
    if key not in _CACHE:
        meta, per_core = _preprocess(np.asarray(src, np.int64),
                                     np.asarray(dst, np.int64))
        nc = _build(meta)
        _CACHE[key] = (meta, per_core, nc)
    return _CACHE[key]


def _in_maps(per_core, feature, W0, b0, Wc, Wl, bl):
    ls = np.arange(1, LAYERS + 1, dtype=np.float32)
    beta = np.log(LAMBDA / ls + 1.0)
    pall = np.zeros((F, LAYERS * F), np.float32)
    eye = np.eye(F, dtype=np.float32)
    for l in range(LAYERS):
        pall[:, l * F:(l + 1) * F] = (1.0 - beta[l]) * eye + beta[l] * Wc[l]
    w0t = np.ascontiguousarray(
        W0.reshape(4, 128, F).transpose(1, 0, 2)).astype(np.float32)
    b0t = np.tile(b0[None, :], (128, 1)).astype(np.float32)
    blt = np.tile(bl[None, :], (128, 1)).astype(np.float32)
    ident = np.eye(128, dtype=np.float32)
    maps = []
    for c in range(C):
        featT = np.zeros((FIN, NPC), np.float32)
        featT[:, :RPC] = feature[c * RPC:(c + 1) * RPC].T
        pc = per_core[c]
        maps.append(dict(featT=featT, w0t=w0t, b0t=b0t, pall=pall,
                         wlt=np.ascontiguousarray(Wl, dtype=np.float32),
                         blt=blt, dinv_t=pc["dinv_t"], dinv1_t=pc["dinv1_t"],
                         idx_main=pc["idx_main"], idx_v=pc["idx_v"],
                         ident=ident))
    return maps


def kernel(feature, W0, b0, Wc, Wl, bl, src, dst):
    feature = np.asarray(feature, np.float32)
    meta, per_core, nc = _get_program(np.asarray(src), np.asarray(dst))
    maps = _in_maps(per_core, feature, np.asarray(W0, np.float32),
                    np.asarray(b0, np.float32), np.asarray(Wc, np.float32),
                    np.asarray(Wl, np.float32), np.asarray(bl, np.float32))
    res = run_bass_kernel_spmd(nc, maps, core_ids=list(range(C)))
    out = np.empty((N, F), np.float32)
    for c in range(C):
        o = res.results[c]["out_t"]          # [128, NB, F]
        out[c * RPC:(c + 1) * RPC] = o.transpose(1, 0, 2).reshape(NPC, F)[:RPC]
    return out

